# revision 23
# baseline (speedup 1.0000x reference)
"""Trainium2 Bass kernel for GTLayer (graph-transformer layer), 8-core SPMD.

Math (matching the torch-style reference exactly):
  QH = h @ Wq.T + bq ; KH, VH likewise                          [N, F]
  per head hh (raw reshape): q_hh = QH[hh*512:(hh+1)*512].view(N, 32)
  t = q @ k.T * scale ; P = softmax(t * A, axis=-1) ; O = P @ v
  y = concat-heads @ Wo.T + bo
  x = BN1(y + h); out = BN2(x + relu(x@W1.T+c1)@W2.T+c2)

Distribution: a row permutation m~ = s*512+u  <->  m = u*8+s turns every
head-view block into natural-layout slices.  Device d owns score rows m~
in [d*512, (d+1)*512) (i.e. Q feature-slice d), computes S^T tiles
(partition = key m~', free = query m~), multiplies by the host-permuted
A^T block (bf16), exponentiates (scale folded into exp, bf16 out), and
accumulates O^T via [V|1]-augmented matmuls giving softmax denominators
in row 32.  K^T and V are computed redundantly on every device, with the
projection work interleaved into the head loop two heads ahead so head 0
starts as soon as its own K/Q/V slices exist.  An AllToAll (bf16)
re-shards from feature-slices to row-blocks for Wo/BN/FFN, which run in
transposed layout (feature on partitions) so BatchNorm stats are
per-partition sums reduced with a tiny AllReduce.

All matmuls use float32r (or bf16) operands - 4x faster than fp32 on the
PE at moving-dim >= 256.  The attention inner loop is software-pipelined
(score MMs emitted 2 groups ahead of the mask-mul/exp/PV chain) so
TensorE, VectorE and ScalarE overlap instead of taking turns.  A dummy
AllReduce keyed on head-6 data warms the collective stream so the
AllToAll trigger latency (~11.5us cold) is not paid on the critical path.
"""

import sys

sys.path.insert(0, "/opt/trn_rl_repo")

from contextlib import ExitStack

import numpy as np
import ml_dtypes

import concourse.bacc as bacc
import concourse.bass as bass
import concourse.tile as tile
from concourse import mybir
from concourse.bass_utils import run_bass_kernel_spmd

ND = 8          # devices
N = 4096        # nodes
F = 256         # hidden
H = 8           # heads
DH = 32         # head dim
L = N // ND     # 512 rows per device
F2 = 2 * F      # ffn hidden
SCALE = DH ** -0.5
EPS = 1e-5
f32 = mybir.dt.float32
f32r = mybir.dt.float32r
bf16 = mybir.dt.bfloat16

# vecs packing (per-partition scalar columns, [128, NVEC])
VEC_BQ = 0        # bq slice d        (32 rows used)
VEC_BK = 1        # bk halves         (2 cols)
VEC_BO = 3        # bo halves         (2 cols)
VEC_C1 = 5        # c1 quarters       (4 cols)
VEC_C2 = 9        # c2 halves         (2 cols)
VEC_G1 = 11       # g1 halves         (2)
VEC_BE1 = 13      # be1 halves        (2)
VEC_G2 = 15       # g2 halves         (2)
VEC_BE2 = 17      # be2 halves        (2)
VEC_EPS = 19      # EPS constant      (1)
NVEC = 20

_CACHE = {}


def _build(bv_zero: bool, bqk_zero: bool):
    nc = bacc.Bacc("TRN2", target_bir_lowering=False, debug=False,
                   num_devices=ND)

    hT_d = nc.dram_tensor("hT", [F, N], bf16, kind="ExternalInput").ap()
    atp_d = nc.dram_tensor("atp", [N, L], bf16, kind="ExternalInput").ap()
    wqT_d = nc.dram_tensor("wqT", [F, DH], bf16, kind="ExternalInput").ap()
    wkT_d = nc.dram_tensor("wkT", [F, F], bf16, kind="ExternalInput").ap()
    wvT_d = nc.dram_tensor("wvT", [F, F], bf16, kind="ExternalInput").ap()
    woT_d = nc.dram_tensor("woT", [F, F], bf16, kind="ExternalInput").ap()
    w1T_d = nc.dram_tensor("w1T", [F, F2], f32, kind="ExternalInput").ap()
    w2T_d = nc.dram_tensor("w2T", [F2, F], f32, kind="ExternalInput").ap()
    vecs_d = nc.dram_tensor("vecs", [128, NVEC], f32, kind="ExternalInput").ap()
    h1T_d = nc.dram_tensor("h1T", [F, L], f32, kind="ExternalInput").ap()
    if not bv_zero:
        bvrow_d = nc.dram_tensor("bvrow", [1, F], f32,
                                 kind="ExternalInput").ap()
    out_d = nc.dram_tensor("out", [F, L], f32, kind="ExternalOutput").ap()

    # collective staging (DRAM only)
    ot_dram = nc.dram_tensor("ot_stage", [H * DH, L], bf16)
    ya_dram = nc.dram_tensor("ya_stage", [H * DH, L], bf16)
    rs_dram = nc.dram_tensor("rs_stage", [H, 512], f32)
    rs2_dram = nc.dram_tensor("rs2_stage", [H, 512], f32)
    warm_out = nc.dram_tensor("warm_out", [1, 16], f32, addr_space="Shared")
    st1_in = nc.dram_tensor("st1_in", [128, 4], f32)
    st1_out = nc.dram_tensor("st1_out", [128, 4], f32, addr_space="Shared")
    st2_in = nc.dram_tensor("st2_in", [128, 4], f32)
    st2_out = nc.dram_tensor("st2_out", [128, 4], f32, addr_space="Shared")

    groups = [list(range(ND))]

    with tile.TileContext(nc) as tc, ExitStack() as ctx:
        big = ctx.enter_context(tc.tile_pool(name="big", bufs=2))
        res = ctx.enter_context(tc.tile_pool(name="res", bufs=1))
        ps = ctx.enter_context(tc.tile_pool(name="ps", bufs=3, space="PSUM"))
        po = ctx.enter_context(tc.tile_pool(name="po", bufs=2, space="PSUM"))
        pt_pool = ctx.enter_context(tc.tile_pool(name="ptp", bufs=3))
        et_pool = ctx.enter_context(tc.tile_pool(name="etp", bufs=3))
        small = ctx.enter_context(tc.tile_pool(name="small", bufs=2))
        ffn = ctx.enter_context(tc.tile_pool(name="ffn", bufs=1))

        # ---- resident tensors ----
        # K^T per 2-head group: [f%128, f//128, 2*512 cols]
        kt_t = [res.tile([128, 2, 1024], bf16, name=f"kt{i}") for i in range(4)]
        # V natural per head: [keypart, up, s', vd(+ones)]
        v_t = [res.tile([128, 4, H, DH + 1], bf16, name=f"v{i}") for i in range(H)]
        # Q^T slice per head, replicated to 4 partition bands
        qt_t = [res.tile([128, 512], bf16, name=f"q{i}") for i in range(H)]
        vecs = res.tile([128, NVEC], f32)
        wq_sb = res.tile([128, 2 * DH], bf16)
        wk_sb = res.tile([128, 2 * F], bf16)
        wv_sb = res.tile([128, 2 * F], bf16)
        wo_sb = res.tile([128, 2 * F], bf16)
        w1_sb = res.tile([128, 2 * F2], f32r)
        w2_sb = res.tile([128, 4 * F], f32r)
        h1_sb = res.tile([128, 2, L], f32)        # h^T[:, d-block] residual
        sh_sb = res.tile([DH, 512], f32)          # head-7 recip staging

        # ---- h^T first: it gates the whole projection/attention pipeline
        ht = [big.tile([128, N], bf16, tag="ht", name=f"ht{i}") for i in range(2)]
        for gc in range(2):
            nc.sync.dma_start(out=ht[gc],
                              in_=hT_d[gc * 128:(gc + 1) * 128, :])
        nc.sync.dma_start(out=vecs, in_=vecs_d)
        nc.vector.memset(sh_sb, 1.0)
        for i in range(H):
            nc.vector.memset(v_t[i][:, :, :, DH:DH + 1], 1.0)
        for gc in range(2):
            nc.sync.dma_start(out=wq_sb[:, gc * DH:(gc + 1) * DH],
                              in_=wqT_d[gc * 128:(gc + 1) * 128, :])
            nc.sync.dma_start(out=wk_sb[:, gc * F:(gc + 1) * F],
                              in_=wkT_d[gc * 128:(gc + 1) * 128, :])
            nc.sync.dma_start(out=wv_sb[:, gc * F:(gc + 1) * F],
                              in_=wvT_d[gc * 128:(gc + 1) * 128, :])
            nc.sync.dma_start(out=h1_sb[:, gc, :],
                              in_=h1T_d[gc * 128:(gc + 1) * 128, :])
        if not bv_zero:
            bvb = res.tile([128, F], f32)
            nc.sync.dma_start(out=bvb, in_=bvrow_d.to_broadcast([128, F]))

        # ---- A^T tiles, bf16, resident (band-major order) ----
        at_t = [big.tile([128, 8, 512], bf16, tag="at", bufs=4,
                         name=f"at{i}") for i in range(4)]

        # ---- projection emitters (interleaved into the head loop) ----
        def emit_q(nck):
            pq = ps.tile([128, 1024], f32, tag="ps")
            for gc in range(2):
                nc.tensor.matmul(pq[0:DH, 0:512],
                                 lhsT=wq_sb[:, gc * DH:(gc + 1) * DH],
                                 rhs=ht[gc][:, nck * 512:(nck + 1) * 512],
                                 start=(gc == 0), stop=(gc == 1))
            if bqk_zero:
                nc.scalar.activation(qt_t[nck][0:DH, :], pq[0:DH, 0:512],
                                     mybir.ActivationFunctionType.Copy)
            else:
                nc.vector.tensor_scalar_add(qt_t[nck][0:DH, :],
                                            pq[0:DH, 0:512],
                                            vecs[0:DH, VEC_BQ:VEC_BQ + 1])
            for band in range(1, 4):
                nc.sync.dma_start(out=qt_t[nck][band * DH:(band + 1) * DH, :],
                                  in_=qt_t[nck][0:DH, :])

        def emit_k(ncs, hfs=(0, 1)):
            for hf in hfs:
                pk = ps.tile([128, 1024], f32, tag="ps")
                for half in range(2):
                    for gc in range(2):
                        nc.tensor.matmul(
                            pk[:, half * 512:(half + 1) * 512],
                            lhsT=wk_sb[:, gc * F + hf * 128:
                                       gc * F + (hf + 1) * 128],
                            rhs=ht[gc][:, (ncs * 2 + half) * 512:
                                       (ncs * 2 + half + 1) * 512],
                            start=(gc == 0), stop=(gc == 1))
                if bqk_zero:
                    nc.scalar.activation(kt_t[ncs][:, hf, :], pk,
                                         mybir.ActivationFunctionType.Copy)
                else:
                    nc.vector.tensor_scalar_add(
                        kt_t[ncs][:, hf, :], pk,
                        vecs[:, VEC_BK + hf:VEC_BK + hf + 1])

        def emit_v(nt):
            pv = ps.tile([128, 1024], f32, tag="ps")
            for gc in range(2):
                nc.tensor.matmul(pv[:, 0:F],
                                 lhsT=ht[gc][:, nt * 128:(nt + 1) * 128],
                                 rhs=wv_sb[:, gc * F:(gc + 1) * F],
                                 start=(gc == 0), stop=(gc == 1))
            src = pv[:, 0:F].rearrange("p (s c) -> p s c", c=DH)
            dst = v_t[nt // 4][:, nt % 4, :, 0:DH]
            if bv_zero:
                nc.scalar.activation(dst, src,
                                     mybir.ActivationFunctionType.Copy)
            else:
                nc.vector.tensor_add(
                    dst, src, bvb.rearrange("p (s c) -> p s c", c=DH))

        # head 0 needs Q0, K ncs0, V nt0-3 before the loop
        emit_q(0)
        emit_k(0)
        for nt in range(4):
            emit_v(nt)

        # A^T loads go on the DMA queues after head-0's critical small DMAs
        for j in range(32):
            hfj, upj, bj = j // 16, (j % 16) // 4, j % 4
            p_tile = hfj * 16 + bj * 4 + upj
            nc.sync.dma_start(
                out=at_t[j // 8][:, j % 8, :],
                in_=atp_d[p_tile * 128:(p_tile + 1) * 128, :])

        # tail-only weights load last; they drain during attention
        for gc in range(2):
            nc.sync.dma_start(out=wo_sb[:, gc * F:(gc + 1) * F],
                              in_=woT_d[gc * 128:(gc + 1) * 128, :])
            nc.sync.dma_start(out=w1_sb[:, gc * F2:(gc + 1) * F2],
                              in_=w1T_d[gc * 128:(gc + 1) * 128, :].bitcast(f32r))
        for kc in range(4):
            nc.sync.dma_start(out=w2_sb[:, kc * F:(kc + 1) * F],
                              in_=w2T_d[kc * 128:(kc + 1) * 128, :].bitcast(f32r))

        # ---- attention: one flat software-pipelined loop across heads ----
        # (scores run 2 groups ahead of the mask/exp/PV chain, across head
        # boundaries, so the PE never drains and HAM stays un-throttled)
        psp_store = {}
        pso_h = {}

        def emit_scores(idx):
            h, g = divmod(idx, 8)
            hf, up = g // 4, g % 4
            psp2 = []
            for pair in range(2):
                psp = ps.tile([128, 1024], f32, tag="ps")
                psp2.append(psp)
                for bi in range(2):
                    band = pair * 2 + bi
                    nc.tensor.matmul(
                        psp[:, bi * 512:(bi + 1) * 512],
                        lhsT=kt_t[h // 2][band * DH:(band + 1) * DH, hf,
                                          (h % 2) * 512 + up * 128:
                                          (h % 2) * 512 + (up + 1) * 128],
                        rhs=qt_t[h][band * DH:(band + 1) * DH, :],
                        start=True, stop=True,
                        tile_position=(band * DH, 0))
            psp_store[idx] = psp2

        def emit_mask_exp(idx):
            h, g = divmod(idx, 8)
            pt = pt_pool.tile([128, 4, 512], f32, tag="pt")
            psp2 = psp_store.pop(idx)
            for pair in range(2):
                j0 = g * 4 + pair * 2
                nc.vector.tensor_mul(
                    pt[:, pair * 2:pair * 2 + 2, :],
                    psp2[pair].rearrange("p (b c) -> p b c", b=2),
                    at_t[j0 // 8][:, j0 % 8:j0 % 8 + 2, :])
            et = et_pool.tile([128, 4, 512], bf16, tag="et")
            nc.scalar.activation(et, pt,
                                 mybir.ActivationFunctionType.Exp,
                                 scale=SCALE)
            return et

        def emit_pv(idx, et):
            h, g = divmod(idx, 8)
            hf, up = g // 4, g % 4
            pso = pso_h[h]
            for pair in range(2):
                for bi in range(2):
                    band = pair * 2 + bi
                    sp = hf * 4 + band
                    nc.tensor.matmul(
                        pso[0:DH + 1, :],
                        lhsT=v_t[h][:, up, sp, :],
                        rhs=et[:, pair * 2 + bi, :],
                        start=(g == 0 and pair == 0 and bi == 0),
                        stop=(g == 7 and pair == 1 and bi == 1))

        def export_head(hh):
            pso = pso_h.pop(hh)
            if hh < H - 1:
                # transposed reciprocal via DRAM roundtrip (off critical path)
                dsb = small.tile([1, 512], f32, tag="dsb", bufs=2)
                nc.scalar.activation(dsb, pso[DH:DH + 1, :],
                                     mybir.ActivationFunctionType.Copy)
                nc.sync.dma_start(out=rs_dram.ap()[hh:hh + 1, :], in_=dsb)
                den128 = small.tile([128, 4], f32, tag="den", bufs=2)
                nc.sync.dma_start(
                    out=den128,
                    in_=rs_dram.ap()[hh:hh + 1, :].rearrange(
                        "o (p j) -> (o p) j", p=128))
                r128 = small.tile([128, 4], f32, tag="rec", bufs=2)
                nc.vector.reciprocal(r128, den128)
                nc.sync.dma_start(
                    out=rs2_dram.ap()[hh:hh + 1, :].rearrange(
                        "o (p j) -> (o p) j", p=128),
                    in_=r128)
                rb = small.tile([DH, 512], f32, tag="rb", bufs=2)
                nc.sync.dma_start(
                    out=rb,
                    in_=rs2_dram.ap()[hh:hh + 1, :].to_broadcast([DH, 512]))
            else:
                # head 7 sits on the critical path into the AllToAll:
                # all-DVE reciprocal + partition-broadcast, no DRAM hops
                nc.vector.reciprocal(sh_sb[0:1, :], pso[DH:DH + 1, :])
                rb = small.tile([DH, 512], f32, tag="rb", bufs=2)
                nc.vector.stream_shuffle(rb, sh_sb, mask=[0] * 32)
            on = small.tile([DH, 512], bf16, tag="on", bufs=2)
            nc.vector.tensor_mul(on, pso[0:DH, :], rb)
            nc.sync.dma_start(out=ot_dram.ap()[hh * DH:(hh + 1) * DH, :],
                              in_=on)

        emit_scores(0)
        emit_scores(1)
        for idx in range(64):
            hh, g = divmod(idx, 8)
            if g == 0:
                pso_h[hh] = po.tile([128, 512], f32, tag="po",
                                    name=f"pso{hh}")
            if idx + 2 < 64:
                emit_scores(idx + 2)
            et = emit_mask_exp(idx)
            emit_pv(idx, et)
            # interleave projection work for head hh+1
            if hh + 1 < H:
                if g == 1:
                    emit_v(4 * (hh + 1))
                    emit_v(4 * (hh + 1) + 1)
                elif g == 3:
                    emit_v(4 * (hh + 1) + 2)
                    emit_v(4 * (hh + 1) + 3)
                elif g == 4:
                    emit_q(hh + 1)
                elif g == 5 and (hh + 1) % 2 == 0:
                    emit_k((hh + 1) // 2)
            if g == 7:
                export_head(hh)
                if hh in (2, 4, 6):
                    # keep the collective stream warm so the AllToAll
                    # trigger latency (~11.5us cold, ~1.2us warm) is hidden
                    nc.gpsimd.collective_compute(
                        "AllReduce", mybir.AluOpType.add,
                        replica_groups=groups,
                        ins=[rs_dram.ap()[hh:hh + 1, 0:16]],
                        outs=[warm_out.ap()])

        # ---- exchange to row-blocks (bf16 payload) ----
        nc.gpsimd.collective_compute(
            "AllToAll", mybir.AluOpType.bypass, replica_groups=groups,
            ins=[ot_dram.ap()], outs=[ya_dram.ap()])

        # PE keep-warm filler while the AllToAll completes (results unused)
        for _jk in range(48):
            pj = ps.tile([128, 512], f32, tag="ps", name=f"pj{_jk}")
            nc.tensor.matmul(pj, lhsT=wk_sb[:, 0:128], rhs=ht[0][:, 0:512],
                             start=True, stop=True)

        yt = [ffn.tile([128, L], bf16, tag=f"yt{i}", name=f"yt{i}") for i in range(2)]
        for gc in range(2):
            nc.sync.dma_start(out=yt[gc],
                              in_=ya_dram.ap()[gc * 128:(gc + 1) * 128, :])

        # ---- Wo + bias + residual -> x1 ; BN1 stats (fused) ----
        x1 = [ffn.tile([128, L], f32, tag=f"x1{i}", name=f"x1{i}") for i in range(2)]
        stat_in = ffn.tile([128, 4], f32, tag="stat")
        for fo in range(2):
            py = ps.tile([128, 1024], f32, tag="ps")
            for gc in range(2):
                nc.tensor.matmul(
                    py[:, 0:512],
                    lhsT=wo_sb[:, gc * F + fo * 128: gc * F + (fo + 1) * 128],
                    rhs=yt[gc],
                    start=(gc == 0), stop=(gc == 1))
            nc.vector.scalar_tensor_tensor(
                x1[fo], py[:, 0:512], vecs[:, VEC_BO + fo:VEC_BO + fo + 1],
                h1_sb[:, fo, :],
                op0=mybir.AluOpType.add, op1=mybir.AluOpType.add,
                accum_out=stat_in[:, fo:fo + 1])
            sq = small.tile([128, 512], f32, tag="sq", bufs=1)
            nc.scalar.activation(sq, x1[fo],
                                 mybir.ActivationFunctionType.Square,
                                 accum_out=stat_in[:, 2 + fo:3 + fo])
        nc.vector.tensor_scalar_mul(stat_in, stat_in, 1.0 / N)
        nc.sync.dma_start(out=st1_in.ap(), in_=stat_in)
        nc.gpsimd.collective_compute(
            "AllReduce", mybir.AluOpType.add, replica_groups=groups,
            ins=[st1_in.ap()], outs=[st1_out.ap()])
        # PE keep-warm filler while the stats AllReduce completes
        for _jk in range(12):
            pj = ps.tile([128, 512], f32, tag="ps", name=f"pj2{_jk}")
            nc.tensor.matmul(pj, lhsT=wk_sb[:, 0:128], rhs=ht[0][:, 0:512],
                             start=True, stop=True)
        st1 = ffn.tile([128, 4], f32, tag="st1")
        nc.sync.dma_start(out=st1, in_=st1_out.ap())

        def bn_affine(st, vg, vbe, tagp):
            """affine coeffs a,b [128,2] from [sumx(2), sumx2(2)] cols."""
            musq = small.tile([128, 2], f32, tag=tagp + "msq", bufs=1)
            nc.scalar.activation(musq, st[:, 0:2],
                                 mybir.ActivationFunctionType.Square)
            var = small.tile([128, 2], f32, tag=tagp + "var", bufs=1)
            nc.vector.scalar_tensor_tensor(
                var, musq, -1.0, st[:, 2:4],
                op0=mybir.AluOpType.mult, op1=mybir.AluOpType.add)
            sd = small.tile([128, 2], f32, tag=tagp + "sd", bufs=1)
            nc.scalar.activation(sd, var, mybir.ActivationFunctionType.Sqrt,
                                 bias=vecs[:, VEC_EPS:VEC_EPS + 1])
            rv = small.tile([128, 2], f32, tag=tagp + "rv", bufs=1)
            nc.vector.reciprocal(rv, sd)
            a = small.tile([128, 2], f32, tag=tagp + "a", bufs=1)
            nc.vector.tensor_mul(a, vecs[:, vg:vg + 2], rv)
            b = small.tile([128, 2], f32, tag=tagp + "b", bufs=1)
            nc.vector.tensor_mul(b, st[:, 0:2], a)
            nc.vector.tensor_sub(b, vecs[:, vbe:vbe + 2], b)
            return a, b

        a1, b1 = bn_affine(st1, VEC_G1, VEC_BE1, "p1")
        x2 = [ffn.tile([128, L], f32r, tag=f"x2{i}", name=f"x2{i}") for i in range(2)]
        for hfi in range(2):
            nc.vector.tensor_scalar(x2[hfi], x1[hfi], a1[:, hfi:hfi + 1],
                                    b1[:, hfi:hfi + 1],
                                    op0=mybir.AluOpType.mult,
                                    op1=mybir.AluOpType.add)

        # ---- FFN ----
        za = [ffn.tile([128, L], f32r,
                       tag=("za%d" % i if i < 2 else "x1%d" % (i - 2)),
                       name=f"za{i}") for i in range(4)]
        for f2t in range(4):
            pz = ps.tile([128, 1024], f32, tag="ps")
            for gc in range(2):
                nc.tensor.matmul(
                    pz[:, 0:512],
                    lhsT=w1_sb[:, gc * F2 + f2t * 128: gc * F2 + (f2t + 1) * 128],
                    rhs=x2[gc],
                    start=(gc == 0), stop=(gc == 1))
            nc.scalar.activation(za[f2t], pz[:, 0:512],
                                 mybir.ActivationFunctionType.Relu,
                                 bias=vecs[:, VEC_C1 + f2t:VEC_C1 + f2t + 1])

        x3 = [ffn.tile([128, L], f32, tag=f"x3{i}", name=f"x3{i}") for i in range(2)]
        stat2 = ffn.tile([128, 4], f32, tag="stat2")
        for fo in range(2):
            p2 = ps.tile([128, 1024], f32, tag="ps")
            for kc in range(4):
                nc.tensor.matmul(
                    p2[:, 0:512],
                    lhsT=w2_sb[:, kc * F + fo * 128: kc * F + (fo + 1) * 128],
                    rhs=za[kc],
                    start=(kc == 0), stop=(kc == 3))
            nc.vector.scalar_tensor_tensor(
                x3[fo], p2[:, 0:512], vecs[:, VEC_C2 + fo:VEC_C2 + fo + 1],
                x2[fo],
                op0=mybir.AluOpType.add, op1=mybir.AluOpType.add,
                accum_out=stat2[:, fo:fo + 1])
            sq2 = small.tile([128, 512], f32, tag="sq", bufs=1)
            nc.scalar.activation(sq2, x3[fo],
                                 mybir.ActivationFunctionType.Square,
                                 accum_out=stat2[:, 2 + fo:3 + fo])
        nc.vector.tensor_scalar_mul(stat2, stat2, 1.0 / N)
        nc.sync.dma_start(out=st2_in.ap(), in_=stat2)
        nc.gpsimd.collective_compute(
            "AllReduce", mybir.AluOpType.add, replica_groups=groups,
            ins=[st2_in.ap()], outs=[st2_out.ap()])
        st2 = ffn.tile([128, 4], f32, tag="st2")
        nc.sync.dma_start(out=st2, in_=st2_out.ap())

        a2, b2 = bn_affine(st2, VEC_G2, VEC_BE2, "p2")
        for hfi in range(2):
            xo = small.tile([128, 512], f32, tag="xo", bufs=2)
            nc.vector.tensor_scalar(xo, x3[hfi], a2[:, hfi:hfi + 1],
                                    b2[:, hfi:hfi + 1],
                                    op0=mybir.AluOpType.mult,
                                    op1=mybir.AluOpType.add)
            nc.sync.dma_start(out=out_d[hfi * 128:(hfi + 1) * 128, :], in_=xo)

    nc.compile()
    return nc


def _get_nc(bv_zero, bqk_zero):
    key = (bv_zero, bqk_zero)
    if key not in _CACHE:
        _CACHE[key] = _build(bv_zero, bqk_zero)
    return _CACHE[key]


def kernel(A, h, Wq, bq, Wk, bk, Wv, bv, Wo, bo, W1, c1, W2, c2,
           g1, be1, g2, be2):
    A = np.asarray(A, np.float32)
    h = np.asarray(h, np.float32)

    idx = np.arange(N)
    perm = (idx % L) * H + idx // L        # m~ -> m
    Ap = A[np.ix_(perm, perm)]
    ApT = np.ascontiguousarray(Ap.T).astype(ml_dtypes.bfloat16)  # [m~', m~]
    hT = np.ascontiguousarray(h.T)
    hTb = hT.astype(ml_dtypes.bfloat16)

    wqT = np.ascontiguousarray(
        np.asarray(Wq, np.float32).T.astype(ml_dtypes.bfloat16))
    wkT = np.ascontiguousarray(
        np.asarray(Wk, np.float32).T.astype(ml_dtypes.bfloat16))
    wvT = np.ascontiguousarray(
        np.asarray(Wv, np.float32).T.astype(ml_dtypes.bfloat16))
    woT = np.ascontiguousarray(
        np.asarray(Wo, np.float32).T.astype(ml_dtypes.bfloat16))
    w1T = np.ascontiguousarray(np.asarray(W1, np.float32).T)
    w2T = np.ascontiguousarray(np.asarray(W2, np.float32).T)

    bv_zero = not np.any(np.asarray(bv))
    bqk_zero = (not np.any(np.asarray(bq))) and (not np.any(np.asarray(bk)))
    nc = _get_nc(bv_zero, bqk_zero)

    def halves(v):
        return np.asarray(v, np.float32).reshape(2, 128).T  # [128, 2]

    in_maps = []
    for d in range(ND):
        vecs = np.zeros((128, NVEC), np.float32)
        vecs[0:DH, VEC_BQ] = np.asarray(bq, np.float32)[d * DH:(d + 1) * DH]
        vecs[:, VEC_BK:VEC_BK + 2] = halves(bk)
        vecs[:, VEC_BO:VEC_BO + 2] = halves(bo)
        vecs[:, VEC_C1:VEC_C1 + 4] = np.asarray(c1, np.float32).reshape(4, 128).T
        vecs[:, VEC_C2:VEC_C2 + 2] = halves(c2)
        vecs[:, VEC_G1:VEC_G1 + 2] = halves(g1)
        vecs[:, VEC_BE1:VEC_BE1 + 2] = halves(be1)
        vecs[:, VEC_G2:VEC_G2 + 2] = halves(g2)
        vecs[:, VEC_BE2:VEC_BE2 + 2] = halves(be2)
        vecs[:, VEC_EPS] = EPS
        m = {
            "hT": hTb,
            "atp": np.ascontiguousarray(ApT[:, d * L:(d + 1) * L]),
            "wqT": np.ascontiguousarray(wqT[:, d * DH:(d + 1) * DH]),
            "wkT": wkT, "wvT": wvT, "woT": woT, "w1T": w1T, "w2T": w2T,
            "vecs": vecs,
            "h1T": np.ascontiguousarray(hT[:, d * L:(d + 1) * L]),
        }
        if not bv_zero:
            m["bvrow"] = np.asarray(bv, np.float32).reshape(1, F)
        in_maps.append(m)

    res = run_bass_kernel_spmd(nc, in_maps, core_ids=list(range(ND)))
    out = np.concatenate(
        [np.asarray(r["out"]).T for r in res.results], axis=0)
    return out.astype(np.float32)


if __name__ == "__main__":
    pass


# revision 24
# speedup vs baseline: 1.1724x; 1.1724x over previous
"""Trainium2 Bass kernel for GTLayer (graph-transformer layer), 8-core SPMD.

Math (matching the torch-style reference exactly):
  QH = h @ Wq.T + bq ; KH, VH likewise                          [N, F]
  per head hh (raw reshape): q_hh = QH[hh*512:(hh+1)*512].view(N, 32)
  t = q @ k.T * scale ; P = softmax(t * A, axis=-1) ; O = P @ v
  y = concat-heads @ Wo.T + bo
  x = BN1(y + h); out = BN2(x + relu(x@W1.T+c1)@W2.T+c2)

Distribution: a row permutation m~ = s*512+u  <->  m = u*8+s turns every
head-view block into natural-layout slices.  Device d owns score rows m~
in [d*512, (d+1)*512) (i.e. Q feature-slice d), computes S^T tiles
(partition = key m~', free = query m~), multiplies by the host-permuted
A^T block (bf16), exponentiates (scale folded into exp, bf16 out), and
accumulates O^T via [V|1]-augmented matmuls giving softmax denominators
in row 32.  K^T and V are computed redundantly on every device, with the
projection work interleaved into the head loop two heads ahead so head 0
starts as soon as its own K/Q/V slices exist.  An AllToAll (bf16)
re-shards from feature-slices to row-blocks for Wo/BN/FFN, which run in
transposed layout (feature on partitions) so BatchNorm stats are
per-partition sums reduced with a tiny AllReduce.

All matmuls use float32r (or bf16) operands - 4x faster than fp32 on the
PE at moving-dim >= 256.  The attention inner loop is software-pipelined
(score MMs emitted 2 groups ahead of the mask-mul/exp/PV chain) so
TensorE, VectorE and ScalarE overlap instead of taking turns.  A dummy
AllReduce keyed on head-6 data warms the collective stream so the
AllToAll trigger latency (~11.5us cold) is not paid on the critical path.
"""

import sys

sys.path.insert(0, "/opt/trn_rl_repo")

from contextlib import ExitStack

import numpy as np
import ml_dtypes

import concourse.bacc as bacc
import concourse.bass as bass
import concourse.tile as tile
from concourse import mybir
from concourse.bass_utils import run_bass_kernel_spmd

ND = 8          # devices
N = 4096        # nodes
F = 256         # hidden
H = 8           # heads
DH = 32         # head dim
L = N // ND     # 512 rows per device
F2 = 2 * F      # ffn hidden
SCALE = DH ** -0.5
EPS = 1e-5
f32 = mybir.dt.float32
f32r = mybir.dt.float32r
bf16 = mybir.dt.bfloat16

# vecs packing (per-partition scalar columns, [128, NVEC])
VEC_BQ = 0        # bq slice d        (32 rows used)
VEC_BK = 1        # bk halves         (2 cols)
VEC_BO = 3        # bo halves         (2 cols)
VEC_C1 = 5        # c1 quarters       (4 cols)
VEC_C2 = 9        # c2 halves         (2 cols)
VEC_G1 = 11       # g1 halves         (2)
VEC_BE1 = 13      # be1 halves        (2)
VEC_G2 = 15       # g2 halves         (2)
VEC_BE2 = 17      # be2 halves        (2)
VEC_EPS = 19      # EPS constant      (1)
NVEC = 20

_CACHE = {}


def _build(bv_zero: bool, bqk_zero: bool):
    nc = bacc.Bacc("TRN2", target_bir_lowering=False, debug=False,
                   num_devices=ND)

    hT_d = nc.dram_tensor("hT", [F, N], bf16, kind="ExternalInput").ap()
    atp_d = nc.dram_tensor("atp", [N, L], bf16, kind="ExternalInput").ap()
    wqT_d = nc.dram_tensor("wqT", [F, DH], bf16, kind="ExternalInput").ap()
    wkT_d = nc.dram_tensor("wkT", [F, F], bf16, kind="ExternalInput").ap()
    wvT_d = nc.dram_tensor("wvT", [F, F], bf16, kind="ExternalInput").ap()
    woT_d = nc.dram_tensor("woT", [F, F], bf16, kind="ExternalInput").ap()
    w1T_d = nc.dram_tensor("w1T", [F, F2], f32, kind="ExternalInput").ap()
    w2T_d = nc.dram_tensor("w2T", [F2, F], f32, kind="ExternalInput").ap()
    vecs_d = nc.dram_tensor("vecs", [128, NVEC], f32, kind="ExternalInput").ap()
    h1T_d = nc.dram_tensor("h1T", [F, L], f32, kind="ExternalInput").ap()
    if not bv_zero:
        bvrow_d = nc.dram_tensor("bvrow", [1, F], f32,
                                 kind="ExternalInput").ap()
    out_d = nc.dram_tensor("out", [F, L], f32, kind="ExternalOutput").ap()

    # collective staging (DRAM only)
    ot_dram = nc.dram_tensor("ot_stage", [H * DH, L], bf16)
    ya_dram = nc.dram_tensor("ya_stage", [H * DH, L], bf16)
    rs_dram = nc.dram_tensor("rs_stage", [H, 512], f32)
    rs2_dram = nc.dram_tensor("rs2_stage", [H, 512], f32)
    warm_out = nc.dram_tensor("warm_out", [1, 16], f32, addr_space="Shared")
    st1_in = nc.dram_tensor("st1_in", [128, 4], f32)
    st1_out = nc.dram_tensor("st1_out", [128, 4], f32, addr_space="Shared")
    st2_in = nc.dram_tensor("st2_in", [128, 4], f32)
    st2_out = nc.dram_tensor("st2_out", [128, 4], f32, addr_space="Shared")

    groups = [list(range(ND))]

    with tile.TileContext(nc) as tc, ExitStack() as ctx:
        big = ctx.enter_context(tc.tile_pool(name="big", bufs=2))
        res = ctx.enter_context(tc.tile_pool(name="res", bufs=1))
        ps = ctx.enter_context(tc.tile_pool(name="ps", bufs=3, space="PSUM"))
        po = ctx.enter_context(tc.tile_pool(name="po", bufs=2, space="PSUM"))
        pt_pool = ctx.enter_context(tc.tile_pool(name="ptp", bufs=3))
        et_pool = ctx.enter_context(tc.tile_pool(name="etp", bufs=3))
        small = ctx.enter_context(tc.tile_pool(name="small", bufs=2))
        ffn = ctx.enter_context(tc.tile_pool(name="ffn", bufs=1))

        # ---- resident tensors ----
        # K^T per 2-head group: [f%128, f//128, 2*512 cols]
        kt_t = [res.tile([128, 2, 1024], bf16, name=f"kt{i}") for i in range(4)]
        # V natural per head: [keypart, up, s', vd(+ones)]
        v_t = [res.tile([128, 4, H, DH + 1], bf16, name=f"v{i}") for i in range(H)]
        # Q^T slice per head, replicated to 4 partition bands
        qt_t = [res.tile([128, 512], bf16, name=f"q{i}") for i in range(H)]
        vecs = res.tile([128, NVEC], f32)
        wq_sb = res.tile([128, 2 * DH], bf16)
        wk_sb = res.tile([128, 2 * F], bf16)
        wv_sb = res.tile([128, 2 * F], bf16)
        wo_sb = res.tile([128, 2 * F], bf16)
        w1_sb = res.tile([128, 2 * F2], f32r)
        w2_sb = res.tile([128, 4 * F], f32r)
        h1_sb = res.tile([128, 2, L], f32)        # h^T[:, d-block] residual
        sh_sb = res.tile([DH, 512], f32)          # head-7 recip staging

        # ---- h^T first: it gates the whole projection/attention pipeline
        ht = [big.tile([128, N], bf16, tag="ht", name=f"ht{i}") for i in range(2)]
        for gc in range(2):
            nc.sync.dma_start(out=ht[gc],
                              in_=hT_d[gc * 128:(gc + 1) * 128, :])
        nc.sync.dma_start(out=vecs, in_=vecs_d)
        nc.vector.memset(sh_sb, 1.0)
        for i in range(H):
            nc.vector.memset(v_t[i][:, :, :, DH:DH + 1], 1.0)
        for gc in range(2):
            nc.sync.dma_start(out=wq_sb[:, gc * DH:(gc + 1) * DH],
                              in_=wqT_d[gc * 128:(gc + 1) * 128, :])
            nc.sync.dma_start(out=wk_sb[:, gc * F:(gc + 1) * F],
                              in_=wkT_d[gc * 128:(gc + 1) * 128, :])
            nc.sync.dma_start(out=wv_sb[:, gc * F:(gc + 1) * F],
                              in_=wvT_d[gc * 128:(gc + 1) * 128, :])
            nc.sync.dma_start(out=h1_sb[:, gc, :],
                              in_=h1T_d[gc * 128:(gc + 1) * 128, :])
        if not bv_zero:
            bvb = res.tile([128, F], f32)
            nc.sync.dma_start(out=bvb, in_=bvrow_d.to_broadcast([128, F]))

        # ---- A^T tiles, bf16, resident (band-major order) ----
        at_t = [big.tile([128, 8, 512], bf16, tag="at", bufs=4,
                         name=f"at{i}") for i in range(4)]

        # ---- projection emitters (interleaved into the head loop) ----
        def emit_q(nck):
            pq = ps.tile([128, 1024], f32, tag="ps")
            for gc in range(2):
                nc.tensor.matmul(pq[0:DH, 0:512],
                                 lhsT=wq_sb[:, gc * DH:(gc + 1) * DH],
                                 rhs=ht[gc][:, nck * 512:(nck + 1) * 512],
                                 start=(gc == 0), stop=(gc == 1))
            if bqk_zero:
                nc.scalar.activation(qt_t[nck][0:DH, :], pq[0:DH, 0:512],
                                     mybir.ActivationFunctionType.Copy)
            else:
                nc.vector.tensor_scalar_add(qt_t[nck][0:DH, :],
                                            pq[0:DH, 0:512],
                                            vecs[0:DH, VEC_BQ:VEC_BQ + 1])
            for band in range(1, 4):
                nc.sync.dma_start(out=qt_t[nck][band * DH:(band + 1) * DH, :],
                                  in_=qt_t[nck][0:DH, :])

        def emit_k(ncs, hfs=(0, 1)):
            for hf in hfs:
                pk = ps.tile([128, 1024], f32, tag="ps")
                for half in range(2):
                    for gc in range(2):
                        nc.tensor.matmul(
                            pk[:, half * 512:(half + 1) * 512],
                            lhsT=wk_sb[:, gc * F + hf * 128:
                                       gc * F + (hf + 1) * 128],
                            rhs=ht[gc][:, (ncs * 2 + half) * 512:
                                       (ncs * 2 + half + 1) * 512],
                            start=(gc == 0), stop=(gc == 1))
                if bqk_zero:
                    nc.scalar.activation(kt_t[ncs][:, hf, :], pk,
                                         mybir.ActivationFunctionType.Copy)
                else:
                    nc.vector.tensor_scalar_add(
                        kt_t[ncs][:, hf, :], pk,
                        vecs[:, VEC_BK + hf:VEC_BK + hf + 1])

        def emit_v(nt):
            pv = ps.tile([128, 1024], f32, tag="ps")
            for gc in range(2):
                nc.tensor.matmul(pv[:, 0:F],
                                 lhsT=ht[gc][:, nt * 128:(nt + 1) * 128],
                                 rhs=wv_sb[:, gc * F:(gc + 1) * F],
                                 start=(gc == 0), stop=(gc == 1))
            src = pv[:, 0:F].rearrange("p (s c) -> p s c", c=DH)
            dst = v_t[nt // 4][:, nt % 4, :, 0:DH]
            if bv_zero:
                nc.scalar.activation(dst, src,
                                     mybir.ActivationFunctionType.Copy)
            else:
                nc.vector.tensor_add(
                    dst, src, bvb.rearrange("p (s c) -> p s c", c=DH))

        # head 0 needs Q0, K ncs0, V nt0-3 before the loop
        emit_q(0)
        emit_k(0)
        for nt in range(4):
            emit_v(nt)

        # A^T loads go on the DMA queues after head-0's critical small DMAs
        for j in range(32):
            hfj, upj, bj = j // 16, (j % 16) // 4, j % 4
            p_tile = hfj * 16 + bj * 4 + upj
            nc.sync.dma_start(
                out=at_t[j // 8][:, j % 8, :],
                in_=atp_d[p_tile * 128:(p_tile + 1) * 128, :])

        # tail-only weights load last; they drain during attention
        for gc in range(2):
            nc.sync.dma_start(out=wo_sb[:, gc * F:(gc + 1) * F],
                              in_=woT_d[gc * 128:(gc + 1) * 128, :])
            nc.sync.dma_start(out=w1_sb[:, gc * F2:(gc + 1) * F2],
                              in_=w1T_d[gc * 128:(gc + 1) * 128, :].bitcast(f32r))
        for kc in range(4):
            nc.sync.dma_start(out=w2_sb[:, kc * F:(kc + 1) * F],
                              in_=w2T_d[kc * 128:(kc + 1) * 128, :].bitcast(f32r))

        # ---- attention: one flat software-pipelined loop across heads ----
        # (scores run 2 groups ahead of the mask/exp/PV chain, across head
        # boundaries, so the PE never drains and HAM stays un-throttled)
        psp_store = {}
        pso_h = {}

        def emit_scores(idx):
            h, g = divmod(idx, 8)
            hf, up = g // 4, g % 4
            psp2 = []
            for pair in range(2):
                psp = ps.tile([128, 1024], f32, tag="ps")
                psp2.append(psp)
                for bi in range(2):
                    band = pair * 2 + bi
                    nc.tensor.matmul(
                        psp[:, bi * 512:(bi + 1) * 512],
                        lhsT=kt_t[h // 2][band * DH:(band + 1) * DH, hf,
                                          (h % 2) * 512 + up * 128:
                                          (h % 2) * 512 + (up + 1) * 128],
                        rhs=qt_t[h][band * DH:(band + 1) * DH, :],
                        start=True, stop=True,
                        tile_position=(band * DH, 0))
            psp_store[idx] = psp2

        def emit_mask_exp(idx):
            h, g = divmod(idx, 8)
            pt = pt_pool.tile([128, 4, 512], f32, tag="pt")
            psp2 = psp_store.pop(idx)
            for pair in range(2):
                j0 = g * 4 + pair * 2
                nc.vector.tensor_mul(
                    pt[:, pair * 2:pair * 2 + 2, :],
                    psp2[pair].rearrange("p (b c) -> p b c", b=2),
                    at_t[j0 // 8][:, j0 % 8:j0 % 8 + 2, :])
            et = et_pool.tile([128, 4, 512], bf16, tag="et")
            nc.scalar.activation(et, pt,
                                 mybir.ActivationFunctionType.Exp,
                                 scale=SCALE)
            return et

        def emit_pv(idx, et):
            h, g = divmod(idx, 8)
            hf, up = g // 4, g % 4
            if g == 0:
                pso_h[h] = po.tile([128, 512], f32, tag="po", name=f"pso{h}")
            pso = pso_h[h]
            for pair in range(2):
                for bi in range(2):
                    band = pair * 2 + bi
                    sp = hf * 4 + band
                    nc.tensor.matmul(
                        pso[0:DH + 1, :],
                        lhsT=v_t[h][:, up, sp, :],
                        rhs=et[:, pair * 2 + bi, :],
                        start=(g == 0 and pair == 0 and bi == 0),
                        stop=(g == 7 and pair == 1 and bi == 1))

        def export_head(hh):
            pso = pso_h.pop(hh)
            if hh < H - 1:
                # transposed reciprocal via DRAM roundtrip (off critical path)
                dsb = small.tile([1, 512], f32, tag="dsb", bufs=2)
                nc.scalar.activation(dsb, pso[DH:DH + 1, :],
                                     mybir.ActivationFunctionType.Copy)
                nc.sync.dma_start(out=rs_dram.ap()[hh:hh + 1, :], in_=dsb)
                den128 = small.tile([128, 4], f32, tag="den", bufs=2)
                nc.sync.dma_start(
                    out=den128,
                    in_=rs_dram.ap()[hh:hh + 1, :].rearrange(
                        "o (p j) -> (o p) j", p=128))
                r128 = small.tile([128, 4], f32, tag="rec", bufs=2)
                nc.vector.reciprocal(r128, den128)
                nc.sync.dma_start(
                    out=rs2_dram.ap()[hh:hh + 1, :].rearrange(
                        "o (p j) -> (o p) j", p=128),
                    in_=r128)
                rb = small.tile([DH, 512], f32, tag="rb", bufs=2)
                nc.sync.dma_start(
                    out=rb,
                    in_=rs2_dram.ap()[hh:hh + 1, :].to_broadcast([DH, 512]))
            else:
                # head 7 sits on the critical path into the AllToAll:
                # all-DVE reciprocal + partition-broadcast, no DRAM hops
                nc.vector.reciprocal(sh_sb[0:1, :], pso[DH:DH + 1, :])
                rb = small.tile([DH, 512], f32, tag="rb", bufs=2)
                nc.vector.stream_shuffle(rb, sh_sb, mask=[0] * 32)
            on = small.tile([DH, 512], bf16, tag="on", bufs=2)
            nc.vector.tensor_mul(on, pso[0:DH, :], rb)
            nc.sync.dma_start(out=ot_dram.ap()[hh * DH:(hh + 1) * DH, :],
                              in_=on)

        emit_scores(0)
        emit_scores(1)
        for idx in range(64):
            hh, g = divmod(idx, 8)
            if idx + 2 < 64:
                emit_scores(idx + 2)
            et = emit_mask_exp(idx)
            emit_pv(idx, et)
            # interleave projection work for head hh+1
            if hh + 1 < H:
                if g == 1:
                    emit_v(4 * (hh + 1))
                    emit_v(4 * (hh + 1) + 1)
                elif g == 3:
                    emit_v(4 * (hh + 1) + 2)
                    emit_v(4 * (hh + 1) + 3)
                elif g == 4:
                    emit_q(hh + 1)
                elif g == 5 and (hh + 1) % 2 == 0:
                    emit_k((hh + 1) // 2)
            if g == 7:
                export_head(hh)
                if hh in (2, 4, 6):
                    # keep the collective stream warm so the AllToAll
                    # trigger latency (~11.5us cold, ~1.2us warm) is hidden
                    nc.gpsimd.collective_compute(
                        "AllReduce", mybir.AluOpType.add,
                        replica_groups=groups,
                        ins=[rs_dram.ap()[hh:hh + 1, 0:16]],
                        outs=[warm_out.ap()])

        # ---- exchange to row-blocks (bf16 payload) ----
        nc.gpsimd.collective_compute(
            "AllToAll", mybir.AluOpType.bypass, replica_groups=groups,
            ins=[ot_dram.ap()], outs=[ya_dram.ap()])

        # PE keep-warm filler while the AllToAll completes (results unused)
        for _jk in range(48):
            pj = ps.tile([128, 512], f32, tag="ps", name=f"pj{_jk}")
            nc.tensor.matmul(pj, lhsT=wk_sb[:, 0:128], rhs=ht[0][:, 0:512],
                             start=True, stop=True)

        yt = [ffn.tile([128, L], bf16, tag=f"yt{i}", name=f"yt{i}") for i in range(2)]
        for gc in range(2):
            nc.sync.dma_start(out=yt[gc],
                              in_=ya_dram.ap()[gc * 128:(gc + 1) * 128, :])

        # ---- Wo + bias + residual -> x1 ; BN1 stats (fused) ----
        x1 = [ffn.tile([128, L], f32, tag=f"x1{i}", name=f"x1{i}") for i in range(2)]
        stat_in = ffn.tile([128, 4], f32, tag="stat")
        for fo in range(2):
            py = ps.tile([128, 1024], f32, tag="ps")
            for gc in range(2):
                nc.tensor.matmul(
                    py[:, 0:512],
                    lhsT=wo_sb[:, gc * F + fo * 128: gc * F + (fo + 1) * 128],
                    rhs=yt[gc],
                    start=(gc == 0), stop=(gc == 1))
            nc.vector.scalar_tensor_tensor(
                x1[fo], py[:, 0:512], vecs[:, VEC_BO + fo:VEC_BO + fo + 1],
                h1_sb[:, fo, :],
                op0=mybir.AluOpType.add, op1=mybir.AluOpType.add,
                accum_out=stat_in[:, fo:fo + 1])
            sq = small.tile([128, 512], f32, tag="sq", bufs=1)
            nc.scalar.activation(sq, x1[fo],
                                 mybir.ActivationFunctionType.Square,
                                 accum_out=stat_in[:, 2 + fo:3 + fo])
        nc.vector.tensor_scalar_mul(stat_in, stat_in, 1.0 / N)
        nc.sync.dma_start(out=st1_in.ap(), in_=stat_in)
        nc.gpsimd.collective_compute(
            "AllReduce", mybir.AluOpType.add, replica_groups=groups,
            ins=[st1_in.ap()], outs=[st1_out.ap()])
        # PE keep-warm filler while the stats AllReduce completes
        for _jk in range(12):
            pj = ps.tile([128, 512], f32, tag="ps", name=f"pj2{_jk}")
            nc.tensor.matmul(pj, lhsT=wk_sb[:, 0:128], rhs=ht[0][:, 0:512],
                             start=True, stop=True)
        st1 = ffn.tile([128, 4], f32, tag="st1")
        nc.sync.dma_start(out=st1, in_=st1_out.ap())

        def bn_affine(st, vg, vbe, tagp):
            """affine coeffs a,b [128,2] from [sumx(2), sumx2(2)] cols."""
            musq = small.tile([128, 2], f32, tag=tagp + "msq", bufs=1)
            nc.scalar.activation(musq, st[:, 0:2],
                                 mybir.ActivationFunctionType.Square)
            var = small.tile([128, 2], f32, tag=tagp + "var", bufs=1)
            nc.vector.scalar_tensor_tensor(
                var, musq, -1.0, st[:, 2:4],
                op0=mybir.AluOpType.mult, op1=mybir.AluOpType.add)
            sd = small.tile([128, 2], f32, tag=tagp + "sd", bufs=1)
            nc.scalar.activation(sd, var, mybir.ActivationFunctionType.Sqrt,
                                 bias=vecs[:, VEC_EPS:VEC_EPS + 1])
            rv = small.tile([128, 2], f32, tag=tagp + "rv", bufs=1)
            nc.vector.reciprocal(rv, sd)
            a = small.tile([128, 2], f32, tag=tagp + "a", bufs=1)
            nc.vector.tensor_mul(a, vecs[:, vg:vg + 2], rv)
            b = small.tile([128, 2], f32, tag=tagp + "b", bufs=1)
            nc.vector.tensor_mul(b, st[:, 0:2], a)
            nc.vector.tensor_sub(b, vecs[:, vbe:vbe + 2], b)
            return a, b

        a1, b1 = bn_affine(st1, VEC_G1, VEC_BE1, "p1")
        x2 = [ffn.tile([128, L], f32r, tag=f"x2{i}", name=f"x2{i}") for i in range(2)]
        for hfi in range(2):
            nc.vector.tensor_scalar(x2[hfi], x1[hfi], a1[:, hfi:hfi + 1],
                                    b1[:, hfi:hfi + 1],
                                    op0=mybir.AluOpType.mult,
                                    op1=mybir.AluOpType.add)

        # ---- FFN ----
        za = [ffn.tile([128, L], f32r,
                       tag=("za%d" % i if i < 2 else "x1%d" % (i - 2)),
                       name=f"za{i}") for i in range(4)]
        for f2t in range(4):
            pz = ps.tile([128, 1024], f32, tag="ps")
            for gc in range(2):
                nc.tensor.matmul(
                    pz[:, 0:512],
                    lhsT=w1_sb[:, gc * F2 + f2t * 128: gc * F2 + (f2t + 1) * 128],
                    rhs=x2[gc],
                    start=(gc == 0), stop=(gc == 1))
            nc.scalar.activation(za[f2t], pz[:, 0:512],
                                 mybir.ActivationFunctionType.Relu,
                                 bias=vecs[:, VEC_C1 + f2t:VEC_C1 + f2t + 1])

        x3 = [ffn.tile([128, L], f32, tag=f"x3{i}", name=f"x3{i}") for i in range(2)]
        stat2 = ffn.tile([128, 4], f32, tag="stat2")
        for fo in range(2):
            p2 = ps.tile([128, 1024], f32, tag="ps")
            for kc in range(4):
                nc.tensor.matmul(
                    p2[:, 0:512],
                    lhsT=w2_sb[:, kc * F + fo * 128: kc * F + (fo + 1) * 128],
                    rhs=za[kc],
                    start=(kc == 0), stop=(kc == 3))
            nc.vector.scalar_tensor_tensor(
                x3[fo], p2[:, 0:512], vecs[:, VEC_C2 + fo:VEC_C2 + fo + 1],
                x2[fo],
                op0=mybir.AluOpType.add, op1=mybir.AluOpType.add,
                accum_out=stat2[:, fo:fo + 1])
            sq2 = small.tile([128, 512], f32, tag="sq", bufs=1)
            nc.scalar.activation(sq2, x3[fo],
                                 mybir.ActivationFunctionType.Square,
                                 accum_out=stat2[:, 2 + fo:3 + fo])
        nc.vector.tensor_scalar_mul(stat2, stat2, 1.0 / N)
        nc.sync.dma_start(out=st2_in.ap(), in_=stat2)
        nc.gpsimd.collective_compute(
            "AllReduce", mybir.AluOpType.add, replica_groups=groups,
            ins=[st2_in.ap()], outs=[st2_out.ap()])
        st2 = ffn.tile([128, 4], f32, tag="st2")
        nc.sync.dma_start(out=st2, in_=st2_out.ap())

        a2, b2 = bn_affine(st2, VEC_G2, VEC_BE2, "p2")
        for hfi in range(2):
            xo = small.tile([128, 512], f32, tag="xo", bufs=2)
            nc.vector.tensor_scalar(xo, x3[hfi], a2[:, hfi:hfi + 1],
                                    b2[:, hfi:hfi + 1],
                                    op0=mybir.AluOpType.mult,
                                    op1=mybir.AluOpType.add)
            nc.sync.dma_start(out=out_d[hfi * 128:(hfi + 1) * 128, :], in_=xo)

    nc.compile()
    return nc


def _get_nc(bv_zero, bqk_zero):
    key = (bv_zero, bqk_zero)
    if key not in _CACHE:
        _CACHE[key] = _build(bv_zero, bqk_zero)
    return _CACHE[key]


def kernel(A, h, Wq, bq, Wk, bk, Wv, bv, Wo, bo, W1, c1, W2, c2,
           g1, be1, g2, be2):
    A = np.asarray(A, np.float32)
    h = np.asarray(h, np.float32)

    idx = np.arange(N)
    perm = (idx % L) * H + idx // L        # m~ -> m
    Ap = A[np.ix_(perm, perm)]
    ApT = np.ascontiguousarray(Ap.T).astype(ml_dtypes.bfloat16)  # [m~', m~]
    hT = np.ascontiguousarray(h.T)
    hTb = hT.astype(ml_dtypes.bfloat16)

    wqT = np.ascontiguousarray(
        np.asarray(Wq, np.float32).T.astype(ml_dtypes.bfloat16))
    wkT = np.ascontiguousarray(
        np.asarray(Wk, np.float32).T.astype(ml_dtypes.bfloat16))
    wvT = np.ascontiguousarray(
        np.asarray(Wv, np.float32).T.astype(ml_dtypes.bfloat16))
    woT = np.ascontiguousarray(
        np.asarray(Wo, np.float32).T.astype(ml_dtypes.bfloat16))
    w1T = np.ascontiguousarray(np.asarray(W1, np.float32).T)
    w2T = np.ascontiguousarray(np.asarray(W2, np.float32).T)

    bv_zero = not np.any(np.asarray(bv))
    bqk_zero = (not np.any(np.asarray(bq))) and (not np.any(np.asarray(bk)))
    nc = _get_nc(bv_zero, bqk_zero)

    def halves(v):
        return np.asarray(v, np.float32).reshape(2, 128).T  # [128, 2]

    in_maps = []
    for d in range(ND):
        vecs = np.zeros((128, NVEC), np.float32)
        vecs[0:DH, VEC_BQ] = np.asarray(bq, np.float32)[d * DH:(d + 1) * DH]
        vecs[:, VEC_BK:VEC_BK + 2] = halves(bk)
        vecs[:, VEC_BO:VEC_BO + 2] = halves(bo)
        vecs[:, VEC_C1:VEC_C1 + 4] = np.asarray(c1, np.float32).reshape(4, 128).T
        vecs[:, VEC_C2:VEC_C2 + 2] = halves(c2)
        vecs[:, VEC_G1:VEC_G1 + 2] = halves(g1)
        vecs[:, VEC_BE1:VEC_BE1 + 2] = halves(be1)
        vecs[:, VEC_G2:VEC_G2 + 2] = halves(g2)
        vecs[:, VEC_BE2:VEC_BE2 + 2] = halves(be2)
        vecs[:, VEC_EPS] = EPS
        m = {
            "hT": hTb,
            "atp": np.ascontiguousarray(ApT[:, d * L:(d + 1) * L]),
            "wqT": np.ascontiguousarray(wqT[:, d * DH:(d + 1) * DH]),
            "wkT": wkT, "wvT": wvT, "woT": woT, "w1T": w1T, "w2T": w2T,
            "vecs": vecs,
            "h1T": np.ascontiguousarray(hT[:, d * L:(d + 1) * L]),
        }
        if not bv_zero:
            m["bvrow"] = np.asarray(bv, np.float32).reshape(1, F)
        in_maps.append(m)

    res = run_bass_kernel_spmd(nc, in_maps, core_ids=list(range(ND)))
    out = np.concatenate(
        [np.asarray(r["out"]).T for r in res.results], axis=0)
    return out.astype(np.float32)


if __name__ == "__main__":
    pass
